# revision 3
# baseline (speedup 1.0000x reference)
"""MultiHeadAttention TRN2 kernel v3 (B=2, L=2048, D=1024, H=16).

Sharding: 8 cores = 2 batches x 4 head-groups (4 heads each).

Design notes (sim-driven):
  - DMA engines are a serial ~57us pipe (one tile DMA saturates all 16):
    global DMA order matters -> single SP queue in consumption order:
    xk, wk, xv, wv, xq, wq, bq, wo. Weights re-DMA per loop iteration
    (idempotent) so x tiles always lead the pipe.
  - Schedule is PE-bound (~164us of matmul rows, bf16-optimal). Front:
    K proj (all) -> V proj (all) -> Q proj (qc 0,1). Attention runs per
    (qp, head, 16-kt chunk); the remaining Q projections and qp0's
    out-projection are interleaved one small unit per kt slot so PE never
    throttles to ACT's exp rate (1.04us/slot vs 0.85 attention-only).
  - bk dropped (softmax-invariant per-query constant); bv folded into the
    host-side bias (attn rows sum to 1 -> contribution is bv @ Wo.T).
  - PSUM: s-tiles 2x2 banks, pv 1x2 banks, px scratch 2x1 bank = 8 banks.
  - pv released by one Pool copy; normalize (recip + partition_broadcast
    + mul) runs off the critical path on DVE/Pool. Last chunk normalizes
    straight from PSUM into ot (shortest tail).
  - Output partials in bf16 (halves the out-DMA drain; host accumulates
    in f32).
"""

import numpy as np

B, L, D, H = 2, 2048, 1024, 16
HD = 64
NH = H // 4  # heads per core = 4
F = NH * HD  # 256 feats per core
NCORES = 8

XDT = "bf16"
BV_FOLDED = True   # kernel output excludes the bv @ Wo.T constant
OUT_BF16 = True

_CACHE = {}


def _build(xdt_name=XDT, repeat=1, loop_n=None):
    from contextlib import ExitStack
    import contextlib

    import concourse.tile as tile
    from concourse import bacc, mybir

    F32 = mybir.dt.float32
    F32R = mybir.dt.float32r
    BF16 = mybir.dt.bfloat16
    XDT_ = mybir.dt.bfloat16 if xdt_name == "bf16" else F32R
    ODT = BF16 if OUT_BF16 else F32
    AF = mybir.ActivationFunctionType

    KB = D // 128   # 8 contraction blocks
    NQC = L // 512  # 4 query chunks
    NKT = L // 128  # 16 key tiles
    NMT = L // 128  # 16 token tiles (V rows)
    NFB = F // 128  # 2 feature blocks
    NDB = D // 128  # 8 dout blocks

    nc = bacc.Bacc(
        "TRN2", target_bir_lowering=False, debug=False, enable_asserts=False
    )

    xq = nc.dram_tensor("xq", [KB, 128, L], XDT_, kind="ExternalInput").ap()
    xk = nc.dram_tensor("xk", [KB, 128, L], XDT_, kind="ExternalInput").ap()
    xv = nc.dram_tensor("xv", [KB, 128, L], XDT_, kind="ExternalInput").ap()
    wq = nc.dram_tensor("wq", [128, KB, F], XDT_, kind="ExternalInput").ap()
    wk = nc.dram_tensor("wk", [128, KB, F], XDT_, kind="ExternalInput").ap()
    wv = nc.dram_tensor("wv", [128, KB, F], XDT_, kind="ExternalInput").ap()
    wo = nc.dram_tensor("wo", [128, NFB, D], F32R, kind="ExternalInput").ap()
    bq = nc.dram_tensor("bq", [128, NFB], F32, kind="ExternalInput").ap()
    out = nc.dram_tensor("out", [NDB, 128, L], ODT, kind="ExternalOutput").ap()

    with tile.TileContext(nc) as tc, ExitStack() as ctx:
        wp = ctx.enter_context(tc.tile_pool(name="wp", bufs=1))
        xt = ctx.enter_context(tc.tile_pool(name="xt", bufs=24))
        qk = ctx.enter_context(tc.tile_pool(name="qk", bufs=1))
        vpp = ctx.enter_context(tc.tile_pool(name="vpp", bufs=16))
        otp = ctx.enter_context(tc.tile_pool(name="otp", bufs=2))
        ep = ctx.enter_context(tc.tile_pool(name="ep", bufs=4))
        rp = ctx.enter_context(tc.tile_pool(name="rp", bufs=2))
        osp = ctx.enter_context(tc.tile_pool(name="osp", bufs=19))
        # PSUM: 4 + 2 + 2 = 8 banks exactly
        ps = ctx.enter_context(tc.tile_pool(name="ps", bufs=2, space="PSUM"))
        pvp = ctx.enter_context(tc.tile_pool(name="pvp", bufs=1, space="PSUM"))
        px = ctx.enter_context(tc.tile_pool(name="px", bufs=2, space="PSUM"))

        wq_s = wp.tile([128, KB, F], XDT_, tag="wq")
        wk_s = wp.tile([128, KB, F], XDT_, tag="wk")
        wv_s = wp.tile([128, KB, F], XDT_, tag="wv")
        wo_s = wp.tile([128, NFB, D], F32R, tag="wo")
        bq_s = wp.tile([128, NFB], F32, tag="bq")
        ones_s = wp.tile([1, 64], F32R, tag="ones")

        loop_ctx = tc.For_i(0, loop_n, 1) if loop_n else contextlib.nullcontext()
        with loop_ctx:
         for _rep in range(repeat):
            # --- single-queue DMA pipe in exact consumption order.
            # wk first: K-proj kb-matmuls can then trickle as xk arrives ---
            xk_t, xq_t, xv_t = [], [], []
            nc.sync.dma_start(wk_s[:], wk)
            for kb in range(KB):
                t = xt.tile([128, L], XDT_, tag="xt", name=f"xk{kb}")
                nc.sync.dma_start(t[:], xk[kb])
                xk_t.append(t)
            for kb in range(KB):
                t = xt.tile([128, L], XDT_, tag="xt", name=f"xv{kb}")
                nc.sync.dma_start(t[:], xv[kb])
                xv_t.append(t)
            nc.sync.dma_start(wv_s[:], wv)
            for kb in range(KB):
                t = xt.tile([128, L], XDT_, tag="xt", name=f"xq{kb}")
                nc.sync.dma_start(t[:], xq[kb])
                xq_t.append(t)
            nc.sync.dma_start(wq_s[:], wq)
            nc.sync.dma_start(bq_s[:], bq)
            nc.sync.dma_start(wo_s[:], wo)
            nc.vector.tensor_scalar(
                ones_s[:], wk_s[0:1, 0:1, 0].broadcast_to([1, 64]),
                0.0, 1.0, mybir.AluOpType.mult, mybir.AluOpType.add,
            )

            kt_t, qt_t = {}, {}
            vp_t = []

            # --- interleavable PE work units ---
            def kq_proj_units(dst, w_s, b_s, x_t, fb, qc, tag):
                """Return 4 closures of 2 kb-matmuls each (one px group)."""
                state = {}

                def unit(i):
                    def run():
                        if i == 0:
                            state["pa"] = px.tile([128, 512], F32, tag="px",
                                                  name=f"p{tag}{fb}{qc}")
                        pa = state["pa"]
                        for kb in (2 * i, 2 * i + 1):
                            nc.tensor.matmul(
                                pa[:],
                                w_s[:, kb, fb * 128:(fb + 1) * 128],
                                x_t[kb][:, qc * 512:(qc + 1) * 512],
                                start=(kb == 0),
                                stop=(kb == KB - 1),
                            )
                        if i == 3:
                            t = qk.tile([128, 512], BF16, tag=f"{tag}{fb}{qc}")
                            if b_s is None:
                                nc.vector.tensor_copy(t[:], pa[:])
                            else:
                                nc.vector.tensor_scalar_add(
                                    t[:], pa[:], b_s[:, fb:fb + 1])
                            dst[(fb, qc)] = t
                    return run

                return [unit(i) for i in range(4)]

            def v_proj(mt):
                pb = px.tile([128, F], F32, tag="px", name=f"pV{mt}")
                for kb in range(KB):
                    nc.tensor.matmul(
                        pb[:],
                        xv_t[kb][:, mt * 128:(mt + 1) * 128],
                        wv_s[:, kb, :],
                        start=(kb == 0),
                        stop=(kb == KB - 1),
                    )
                v = vpp.tile([128, NH, 65], BF16, tag="vp", name=f"vp{mt}")
                nc.vector.tensor_scalar(
                    v[:, :, 64], wv_s[:, 0:1, 0].broadcast_to([128, NH]),
                    0.0, 1.0, mybir.AluOpType.mult, mybir.AluOpType.add,
                )
                nc.vector.tensor_copy(
                    v[:, :, 0:64],
                    pb[:].rearrange("p (h f) -> p h f", h=NH),
                )
                vp_t.append(v)

            def out_proj_unit(ot, qp, ql, mt, copy_eng=None, dma_eng=None,
                              pool_tag=None):
                def run():
                    qc = qp * 2 + ql
                    pool, tag = pool_tag or (px, "px")
                    po = pool.tile([128, 512], F32, tag=tag,
                                   name=f"po{qp}{ql}{mt}")
                    for fb in range(NFB):
                        nc.tensor.matmul(
                            po[:],
                            wo_s[:, fb, mt * 128:(mt + 1) * 128],
                            ot[:, fb, ql, :],
                            start=(fb == 0),
                            stop=(fb == NFB - 1),
                        )
                    og = osp.tile([128, 512], ODT, tag="og")
                    if copy_eng is None:
                        nc.scalar.activation(og[:], po[:], AF.Copy)
                    else:
                        copy_eng.tensor_copy(og[:], po[:])
                    (dma_eng or nc.sync).dma_start(
                        out[mt][:, qc * 512:(qc + 1) * 512], og[:]
                    )
                return run

            # --- front phase: K proj (all), V proj (all), Q proj qc0,1 of
            # fb0 ... wait: chunk order needs fb0 Q first ---
            for fb in range(NFB):
                for qc in range(NQC):
                    for u in kq_proj_units(kt_t, wk_s, None, xk_t, fb, qc,
                                           "kt"):
                        u()
            for mt in range(NMT):
                v_proj(mt)
            for fb in range(NFB):
                for qc in (0, 1):
                    for u in kq_proj_units(qt_t, wq_s, bq_s, xq_t, fb, qc,
                                           "qt"):
                        u()

            # --- attention: flat software-pipelined slot loop.
            # Slot j emits S(j)+exp(j) and PV(j-LAG); the lag removes the
            # chunk-boundary bubble (PE never waits for exp of the slot it
            # just issued) and gives the pv-release Pool copy time to
            # finish before the next chunk's first PV needs the banks. ---
            LAG = 3
            # pp0 chunks first within each qp so ot[:, fb0] completes
            # halfway through qp1 (enables fb0-partial out-proj); last
            # chunk is hh=0 (writes ot directly -> shortest tail)
            CHUNKS = [(qp, pp, hh) for qp in range(2)
                      for pp, hh in ((0, 1), (0, 0), (1, 1), (1, 0))]
            ots = []
            pv_c = {}

            def normalize(ci):
                qp, pp, hh = CHUNKS[ci]
                ot = ots[qp]
                pv = pv_c.pop(ci)
                last = ci == len(CHUNKS) - 1
                if last:
                    pvsrc = pv
                else:
                    pvsrc = rp.tile([65, 2, 512], F32, tag="pvs")
                    nc.vector.tensor_copy(pvsrc[:], pv[:])
                # broadcast raw rowsums via ones-matmul (moving operand
                # must be produced as rounded f32r -> DVE copy), then
                # reciprocal on the broadcast rows
                rs = rp.tile([1, 2, 512], F32R, tag="rs")
                nc.vector.tensor_copy(rs[:], pvsrc[64:65, :, :])
                dst = (ot[0:64, pp, :, :] if hh == 0 else pvsrc[0:64, :, :])
                for ql in range(2):
                    rb = px.tile([64, 512], F32, tag="px",
                                 name=f"rb{ci}{ql}")
                    nc.tensor.matmul(
                        rb[:], ones_s[:], rs[:, ql, :],
                        start=True, stop=True,
                    )
                    rc = rp.tile([64, 512], F32, tag=f"rc{ql}")
                    nc.vector.reciprocal_approx_fast(rc[:], rb[:])
                    nc.vector.tensor_mul(
                        dst[:, ql, :], pvsrc[0:64, ql, :], rc[:])
                if hh == 1:
                    nc.sync.dma_start(ot[64:128, pp, :, :],
                                      pvsrc[0:64, :, :].bitcast(F32R))

            def pv_step(ci, kt, e):
                qp, pp, hh = CHUNKS[ci]
                h = 2 * pp + hh
                if kt == 0:
                    pv_c[ci] = pvp.tile([65, 2, 512], F32, tag="pv",
                                        name=f"pv{ci}")
                pv = pv_c[ci]
                for ql in range(2):
                    nc.tensor.matmul(
                        pv[:, ql, :],
                        vp_t[kt][:, h, :],
                        e[:, ql, :],
                        start=(kt == 0),
                        stop=(kt == NKT - 1),
                    )
                if kt == NKT - 1:
                    normalize(ci)

            work = []
            wi = 0
            pend = []
            for ci in range(len(CHUNKS)):
                qp, pp, hh = CHUNKS[ci]
                pl, ph = 64 * hh, 64 * (hh + 1)
                if ci % 4 == 0:
                    ot = otp.tile([128, NFB, 2, 512], F32R, tag="ot",
                                  name=f"ot{qp}")
                    ots.append(ot)
                    if qp == 0:
                        for fb in range(NFB):
                            for qc in (2, 3):
                                work.extend(kq_proj_units(
                                    qt_t, wq_s, bq_s, xq_t, fb, qc, "qt"))
                    else:
                        for i in range(2 * NDB):
                            ql, mt = divmod(i, NDB)
                            work.append(out_proj_unit(ots[0], 0, ql, mt, copy_eng=nc.vector))
                for kt in range(NKT):
                    s = ps.tile([128, 2, 512], F32, tag="s",
                                name=f"s{ci}{kt}")
                    for ql in range(2):
                        qc = qp * 2 + ql
                        nc.tensor.matmul(
                            s[:, ql, :],
                            kt_t[(pp, kt // 4)][pl:ph,
                                                (kt % 4) * 128:
                                                (kt % 4 + 1) * 128],
                            qt_t[(pp, qc)][pl:ph, :],
                            start=True,
                            stop=True,
                        )
                    e = ep.tile([128, 2, 512], BF16, tag="e",
                                name=f"e{ci}{kt}")
                    nc.scalar.activation(e[:], s[:], AF.Exp, scale=0.125)
                    pend.append((ci, kt, e))
                    if len(pend) > LAG:
                        pv_step(*pend.pop(0))
                    # normalize of the previous chunk lands at kt~2 (LAG)
                    # and allocates rb from px: qp0's multi-slot projection
                    # groups must not be open across it (PE in-order queue
                    # would deadlock on the px slot), so their units drain
                    # only at kt 4..13
                    drain = (kt in (4, 7, 10, 13)) if qp == 0 else (
                        kt % 2 == 1 and kt >= 3)
                    if drain and wi < len(work):
                        work[wi]()
                        wi += 1
            while pend:
                pv_step(*pend.pop(0))
            while wi < len(work):
                work[wi]()
                wi += 1
            # tail: out-proj qp1. s/pv PSUM pools are idle now -- rotate
            # po across all three pools; staging copies alternate between
            # the (idle) ACT engine and DVE; DMAs alternate queues
            for i in range(2 * NDB):
                ql, mt = divmod(i, NDB)
                out_proj_unit(
                    ots[1], 1, ql, mt,
                    copy_eng=(nc.vector if i % 2 == 0 else None),
                    dma_eng=(nc.sync if i % 2 == 0 else nc.scalar),
                    pool_tag=((px, "px"), (ps, "s"), (pvp, "pv"))[i % 3])()

    nc.compile()
    return nc


def _prep_core(b, g, query, key_, value, Wq, bq, Wk, bk, Wv, bv, Wo,
               xdt_name=XDT):
    """Host-side shard prep for core handling batch b, head group g."""
    import ml_dtypes

    fs = g * F
    f32 = np.float32
    xdt = ml_dtypes.bfloat16 if xdt_name == "bf16" else f32

    def xT(x):
        return np.ascontiguousarray(
            x[b].T.reshape(D // 128, 128, L), dtype=xdt
        )

    def wT(W):
        return np.ascontiguousarray(
            W[fs:fs + F, :].T.reshape(D // 128, 128, F).transpose(1, 0, 2),
            dtype=xdt,
        )

    wos = np.ascontiguousarray(
        Wo[:, fs:fs + F].T.reshape(F // 128, 128, D).transpose(1, 0, 2),
        dtype=f32,
    )
    return {
        "xq": xT(query),
        "xk": xT(key_),
        "xv": xT(value),
        "wq": wT(Wq),
        "wk": wT(Wk),
        "wv": wT(Wv),
        "wo": wos,
        "bq": np.ascontiguousarray(bq[fs:fs + F].reshape(F // 128, 128).T, f32),
    }


def kernel(query, key_, value, Wq, bq, Wk, bk, Wv, bv, Wo, bo):
    import os

    from concourse.bass_utils import run_bass_kernel_spmd

    if "nc" not in _CACHE:
        _CACHE["nc"] = _build()
    nc = _CACHE["nc"]

    args = [np.asarray(a, np.float32) for a in
            (query, key_, value, Wq, bq, Wk, bk, Wv, bv, Wo)]
    in_maps = [_prep_core(c // 4, c % 4, *args) for c in range(NCORES)]
    res = run_bass_kernel_spmd(
        nc, in_maps, core_ids=list(range(NCORES)),
        tmpdir=os.environ.get("BASS_TRACE_DIR") or None,
    )
    globals()["_LAST_EXEC_NS"] = res.exec_time_ns

    final = np.zeros((B, L, D), np.float32)
    for c in range(NCORES):
        o = res.results[c]["out"]  # [8, 128, L] partial out.T blocks
        final[c // 4] += o.reshape(D, L).T.astype(np.float32)
    # bv folded out of the kernel: attn rows sum to 1, so its contribution
    # is the constant vector bv @ Wo.T
    final += np.asarray(bo, np.float32) + (
        np.asarray(bv, np.float32) @ np.asarray(Wo, np.float32).T)
    return final
